# revision 24
# baseline (speedup 1.0000x reference)
"""MoE layer (B=8192, D=2048, H=2048, E=8, top-2) on 8 TRN2 NeuronCores.

Strategy: expert-parallel with host-side routing + PER-PAIR MIXED PRECISION.
kernel() receives the FULL inputs on host, so the dispatch/combine all-to-all
is simply the sharding step:

  1. Gating (0.2% of FLOPs) on host with jax-CPU, bit-matching the
     reference's `x @ gate_W.T + gate_b` -> top_k -> softmax.
  2. Per expert, its T_BF*128 highest-gate-weight (token, expert) pairs run
     in bf16; the rest run in fp8 e4m3 with MatmulPerfMode.DoubleRow (2x PE
     throughput, 157 TF/s). Host-sim rel-err of this split is 1.64e-2 vs
     the 2e-2 gate (plain-fp8 everything would be 3.3e-2 — fails; all-bf16
     is 2x slower at 1.8x the HW time). fp8 scales align the uniform W
     distribution with the e4m3 grid (~15% less quant error than power-of-2
     scaling); the descale folds into the host combine.
  3. bf16 side: core e computes expert e's 1024 pairs in one 1024-row
     segment — zero padding. fp8 side: per-expert remainders are split into
     128-row blocks and best-fit packed into two segments per core
     (SPMD: all cores share one program, so segment shapes must match
     across cores). Leftover fp8 slot rows are filled with antithetically-
     quantized DUPLICATES of the highest-weight fp8 pairs; the combine
     averages each pair's two copies, cancelling ~75% of x-quantization
     error variance for those pairs at zero device cost.
  4. Segment order [fp8, fp8, bf16]: fp8 first tiles are half the bytes, so
     the PE starts earlier (the first call also uses a 256-row K tile to
     shrink the critical first transfers). More than 3 segments loses:
     each segment streams its own W copy and the extra DMA stalls the PE
     (measured). fp32 PSUM accumulate; bf16 output evicted via the vector
     engine (halves y write-back vs fp32).
  5. Host combine: out[b] = sum_k w_k[b] * (Y_pair[row(pair)]/scale(pair)
     + b_{e_k(b)}).
"""

import numpy as np

B, D, H, E, TOPK = 8192, 2048, 2048, 8, 2
NCORES = 8

T_BF = 8          # bf16 blocks (x128 rows) per expert = per core
SX = 24.0         # x fp8 scale (randn -> +-131, e4m3 normal range)
SW = 2976.0       # W fp8 scale (U(+-0.0221) -> +-65.8, grid-aligned)
# Dummies trip the HAM clock monitor AND bridge the PE queue until the
# first real matmul's tiles arrive (~14-16us for the custom fp8 head):
# a >3.4us idle gap after the trip re-throttles the PE to 1.2GHz
# (measured: ~8.8us of slowed matmuls). 72 dummies end ~14.6us.
WARMUP_MM = 72

# test.py flips TRACE to profile HW exec time; grading leaves it False.
TRACE = False
last_exec_time_ns = None
last_trace_path = None


def _routing(x, gate_W, gate_b):
    """Reference-exact gating on jax-CPU: logits -> top_k -> softmax."""
    import jax
    import jax.numpy as jnp

    with jax.default_device(jax.devices("cpu")[0]):
        logits = jnp.asarray(x) @ jnp.asarray(gate_W).T + jnp.asarray(gate_b)
        topk_vals, topk_idx = jax.lax.top_k(logits, TOPK)
        topk_w = jax.nn.softmax(topk_vals, axis=1)
    return np.asarray(topk_idx), np.asarray(topk_w, dtype=np.float32)


def _ok(b):
    # Segment block counts divisible by 3 or 4 keep matmul_tile_kernel's
    # M_TILE at 384/512 (vs 128), avoiding extra W re-streaming DMA.
    return b > 0 and (b % 3 == 0 or b % 4 == 0)


def _pack(block_counts):
    """Pack per-group block counts into identical per-core segments.

    Returns (seg_blocks, pieces) with pieces[g] = [(core, seg, blocks,
    bin_blocks)]; each (core, seg) bin holds at most one group, pieces fill
    bins from the start (partial fills leave slack at the bin end, which
    the dispatch fills with duplicated rows).
    """
    total = sum(block_counts)
    if total == 0:
        return [], [[] for _ in block_counts]
    t0 = -(-total // 8)
    for T in range(t0, t0 + 5):
        schemes = []
        if _ok(T):
            schemes.append([T])
        schemes += [
            [a, T - a] for a in range(T - 1, T // 2 - 1, -1) if _ok(a) and _ok(T - a)
        ]
        for seg_blocks in schemes:
            bins = []
            for c in range(8):
                for j, bcap in enumerate(seg_blocks):
                    bins.append([bcap, c, j])
            pieces = [[] for _ in block_counts]
            feasible = True
            for g in sorted(range(len(block_counts)), key=lambda g: -block_counts[g]):
                rem = block_counts[g]
                while rem > 0 and bins:
                    bins.sort(key=lambda s: -s[0])
                    if rem >= bins[0][0]:
                        pick, take = bins[0], bins[0][0]
                    else:
                        pick = min(
                            (s for s in bins if s[0] >= rem), key=lambda s: s[0]
                        )
                        take = rem
                    pieces[g].append((pick[1], pick[2], take, pick[0]))
                    rem -= take
                    bins.remove(pick)
                if rem > 0:
                    feasible = False
                    break
            if feasible:
                return seg_blocks, pieces
    raise RuntimeError("fp8 packing failed")


def _build_bass(f8_seg_rows, bf_rows):
    """One Bass program, SPMD across cores. Hand-rolled fp8 DoubleRow
    segments first, then the bf16 segment via matmul_tile_kernel. y rows
    follow segment order; bf16 dtype.

    The custom fp8 section fixes what matmul_tile_kernel can't: W8 streams
    exactly once (no per-M-tile re-streaming), and x8 arrives from a
    host-pre-tiled layout (128 descriptors of 2KB per tile instead of 2048
    of 128B), so the fp8 phase stops saturating the DMA queues and stalling
    the PE.
    """
    import concourse.bacc as bacc
    import concourse.mybir as mybir
    import concourse.tile as tile
    from concourse.kernels.tile_matmul import matmul_tile_kernel

    P, KO, NB = 128, D // 128, H // 512
    C8 = sum(f8_seg_rows)
    M8 = C8 // P
    C = C8 + bf_rows
    f8, bf16, f32 = mybir.dt.float8e4, mybir.dt.bfloat16, mybir.dt.float32
    nc = bacc.Bacc("TRN2", target_bir_lowering=False)
    # x8t[p, m, ko, i] = x8[ko*128 + p, m*128 + i] — each SBUF tile
    # [128, KO, 128] is 2KB-contiguous per partition in DRAM
    x8t = nc.dram_tensor("x8t", [P, M8, KO, P], f8, kind="ExternalInput")
    xTb = nc.dram_tensor("xTb", [D, bf_rows], bf16, kind="ExternalInput")
    w8s = [
        nc.dram_tensor(f"w8_{j}", [D, H], f8, kind="ExternalInput")
        for j in range(len(f8_seg_rows))
    ]
    wb = nc.dram_tensor("wb", [D, H], bf16, kind="ExternalInput")
    y = nc.dram_tensor("y", [C, H], bf16, kind="ExternalOutput")
    with tile.TileContext(nc) as tc:
        # PE warm-up: tiny matmuls with no DMA deps run during the initial
        # tile-fill window, tripping the HAM activity monitor (4096-cycle
        # window) so the real matmuls start at 2.4 GHz instead of the cold
        # 1.2 GHz, and bridging the idle gap so it can't re-throttle before
        # the first real matmul.
        with (
            tc.tile_pool(name="warm", bufs=1) as warm,
            tc.tile_pool(name="warmp", bufs=1, space="PSUM") as warmp,
        ):
            wa = warm.tile([128, 128], bf16)
            nc.vector.memset(wa[:], 0.0)
            pts = [
                warmp.tile([128, 128], f32, name=f"wp{i}", tag=f"wp{i}")
                for i in range(4)
            ]
            for i in range(WARMUP_MM):
                nc.tensor.matmul(pts[i % 4][:], wa[:], wa[:], start=True, stop=True)

        # ---- custom fp8 DoubleRow sections, sandwiching the bf16 call:
        # seg0 first (quarter-split head tiles start the PE early), bf16 in
        # the middle (its 512KB first tiles prefetch during seg0 compute),
        # seg1 last (its 128KB y-writes drain ~4us faster than the bf16
        # tile kernel's final 512KB block would).
        with (
            tc.tile_pool(name="w8pool", bufs=2) as wpool,
            tc.tile_pool(name="x8pool", bufs=1) as xpool,
            tc.tile_pool(name="y8pool", bufs=3) as ypool,
            tc.tile_pool(name="ps8", bufs=2, space="PSUM") as pspool,
        ):
            # x tiles become SBUF-resident (2.6MB total), loaded lazily in
            # each segment's first n-sweep. x on the sync queue, steady-state
            # weights on gpsimd, so the streams don't serialize each other.
            xt = [None] * M8
            seg_mg = [sum(s // P for s in f8_seg_rows[:j]) for j in range(len(f8_seg_rows))]

            def f8_segment(j):
                mg = seg_mg[j]
                wr = w8s[j].rearrange("(ko p) h -> p ko h", p=P)
                M = f8_seg_rows[j] // P
                for n in range(NB):
                    if n == 0:
                        xt[mg] = xpool.tile(
                            [P, KO, P], f8, name=f"x8_{mg}", tag=f"x8_{mg}"
                        )
                        if j == 0:
                            # one dma_start = one queue at ~22GB/s, so split
                            # the first x tile into 64KB quarters across
                            # parallel queues
                            for q in range(4):
                                nc.sync.dma_start(
                                    xt[mg][:, 4 * q : 4 * q + 4, :],
                                    x8t[:, mg, 4 * q : 4 * q + 4, :],
                                )
                        else:
                            nc.sync.dma_start(xt[mg][:], x8t[:, mg, :, :])
                    wt = []
                    for g in range(4):  # 4 ko-groups of 4 -> 256KB chunks
                        t = wpool.tile(
                            [P, 4, 512], f8, name=f"w8_{j}_{n}_{g}", tag=f"w8g_{g}"
                        )
                        sl = wr[:, g * 4 : (g + 1) * 4, n * 512 : (n + 1) * 512]
                        if j == 0 and n == 0:
                            # first W group on sync in 64KB quarters: 16
                            # issues land on parallel queues (~3us each)
                            # instead of 4 serial 256KB gpsimd loads
                            for q in range(4):
                                nc.sync.dma_start(
                                    t[:, q : q + 1, :], sl[:, q : q + 1, :]
                                )
                        else:
                            nc.gpsimd.dma_start(t[:], sl)
                        wt.append(t)
                    for m in range(M):
                        if n == 0 and m > 0:
                            xt[mg + m] = xpool.tile(
                                [P, KO, P], f8, name=f"x8_{mg + m}", tag=f"x8_{mg + m}"
                            )
                            nc.sync.dma_start(xt[mg + m][:], x8t[:, mg + m, :, :])
                        ps = pspool.tile([P, 512], f32, tag="ps8")
                        for k2 in range(KO // 2):  # 8 DoubleRow matmuls, K=256
                            ko = 2 * k2
                            nc.tensor.matmul(
                                ps[:],
                                xt[mg + m][:, ko : ko + 2, :],
                                wt[ko // 4][:, ko % 4 : ko % 4 + 2, :],
                                start=(k2 == 0),
                                stop=(k2 == KO // 2 - 1),
                                perf_mode=mybir.MatmulPerfMode.DoubleRow,
                            )
                        yt = ypool.tile([P, 512], bf16, tag="y8")
                        nc.vector.tensor_copy(yt[:], ps[:])
                        nc.sync.dma_start(
                            y[
                                (mg + m) * P : (mg + m + 1) * P,
                                n * 512 : (n + 1) * 512,
                            ],
                            yt[:],
                        )

            f8_segment(0)
            matmul_tile_kernel(
                tc,
                xTb[:, :],
                wb[:],
                y[C8:, :],
                psum_evict_fn=lambda nc, psum, sbuf: nc.vector.tensor_copy(
                    out=sbuf, in_=psum
                ),
            )
            for j in range(1, len(f8_seg_rows)):
                f8_segment(j)
    nc.compile()
    return nc


def _install_profshim():
    """Register the NTFF profile hook trn_boot couldn't (image's antenv lacks
    axon_hooks) and stub the S3 artifact upload. Only needed when TRACE."""
    import sys
    import types

    import antenv

    if "antenv.axon_hooks" not in sys.modules:
        mod = types.ModuleType("antenv.axon_hooks")
        _hook = [None]
        mod.set_axon_ntff_profile_hook = lambda h: _hook.__setitem__(0, h)
        mod.get_axon_ntff_profile_hook = lambda: _hook[0]
        sys.modules["antenv.axon_hooks"] = mod
        antenv.axon_hooks = mod
        from trn_agent_boot.trn_boot import _ntff_profile_via_ctypes

        mod.set_axon_ntff_profile_hook(
            _ntff_profile_via_ctypes("/opt/axon/libaxon_pjrt.so")
        )
    import concourse.bass_utils as _bu

    _bu.upload_artifacts = lambda tmpdir: f"local:{tmpdir}"


def kernel(x, expert_W, expert_b, gate_W, gate_b):
    global last_exec_time_ns, last_trace_path
    import ml_dtypes

    from concourse.bass_utils import run_bass_kernel_spmd

    x = np.asarray(x, dtype=np.float32)
    expert_W = np.asarray(expert_W, dtype=np.float32)
    expert_b = np.asarray(expert_b, dtype=np.float32)
    gate_W = np.asarray(gate_W, dtype=np.float32)
    gate_b = np.asarray(gate_b, dtype=np.float32)

    topk_idx, topk_w = _routing(x, gate_W, gate_b)

    # per-expert token lists by gate weight desc; top T_BF*128 -> bf16
    lists = []
    for e in range(E):
        bb, kk = np.nonzero(topk_idx == e)
        ww = topk_w[bb, kk]
        o = np.argsort(-ww, kind="stable")
        lists.append(bb[o])
        assert len(bb) >= T_BF * 128, f"expert {e} has only {len(bb)} pairs"

    NBF = T_BF * 128
    counts_8 = [len(lists[e]) - NBF for e in range(E)]
    f8_blocks = [-(-n // 128) for n in counts_8]
    f8_seg, f8_pieces = _pack(f8_blocks)
    f8_seg_rows = [b * 128 for b in f8_seg]
    f8_off = np.concatenate([[0], np.cumsum(f8_seg_rows)]).astype(int)
    C8 = int(f8_off[-1])

    bf16 = ml_dtypes.bfloat16
    f8 = ml_dtypes.float8_e4m3
    xb = x.astype(bf16)  # one RTN cast, reused for all bf16 gathers
    x8 = (x * np.float32(SX)).astype(f8)
    # antithetic second quantization for duplicated rows: reflect x about its
    # first quantization so averaging the two copies cancels most of the
    # rounding error
    x8d = ((2.0 * x - x8.astype(np.float32) / np.float32(SX)) * np.float32(SX)).astype(
        f8
    )
    wbq = [np.ascontiguousarray(expert_W[e].T.astype(bf16)) for e in range(E)]
    w8q = [
        np.ascontiguousarray((expert_W[e].T * np.float32(SW)).astype(f8))
        for e in range(E)
    ]

    xT8s = [np.zeros((D, C8), dtype=f8) for _ in range(NCORES)]
    xTbs = [np.empty((D, NBF), dtype=bf16) for _ in range(NCORES)]
    seg8_w = [[0] * len(f8_seg) for _ in range(NCORES)]
    core_of = np.zeros((E, B), dtype=np.int64)
    pos_of = np.zeros((E, B), dtype=np.int64)
    is8_of = np.zeros((E, B), dtype=bool)
    dup_core = np.zeros((E, B), dtype=np.int64)
    dup_pos = np.zeros((E, B), dtype=np.int64)
    has_dup = np.zeros((E, B), dtype=bool)

    for e in range(E):
        toks = lists[e]
        tb, t8 = toks[:NBF], toks[NBF:]
        xTbs[e][:, :] = xb[tb].T  # bf16: expert e lives on core e, full slot
        core_of[e, tb] = e
        pos_of[e, tb] = C8 + np.arange(NBF)
        cum = 0
        dup_used = 0
        for c, j, blk, bin_blk in f8_pieces[e]:
            cap = bin_blk * 128  # fill the whole bin; slack becomes dups
            tkn = t8[cum : cum + blk * 128]
            lo = int(f8_off[j])
            xT8s[c][:, lo : lo + len(tkn)] = x8[tkn].T
            seg8_w[c][j] = e
            core_of[e, tkn] = c
            pos_of[e, tkn] = lo + np.arange(len(tkn))
            is8_of[e, tkn] = True
            slack = cap - len(tkn)
            if slack > 0:
                # fill leftover rows with antithetic duplicates of the
                # highest-weight fp8 pairs; combine averages the two copies
                dsel = t8[dup_used : dup_used + slack]
                if len(dsel):
                    xT8s[c][:, lo + len(tkn) : lo + len(tkn) + len(dsel)] = x8d[dsel].T
                    dup_core[e, dsel] = c
                    dup_pos[e, dsel] = lo + len(tkn) + np.arange(len(dsel))
                    has_dup[e, dsel] = True
                    dup_used += len(dsel)
            cum += blk * 128

    M8 = C8 // 128
    in_maps = []
    for c in range(NCORES):
        # pre-tile x8 for the custom section: [p, m, ko, i] so each SBUF
        # tile is 2KB-contiguous per partition
        x8t = np.ascontiguousarray(
            xT8s[c].reshape(D // 128, 128, M8, 128).transpose(1, 2, 0, 3)
        )
        mm = {"x8t": x8t, "xTb": xTbs[c], "wb": wbq[c]}
        for j in range(len(f8_seg)):
            mm[f"w8_{j}"] = w8q[seg8_w[c][j]]
        in_maps.append(mm)

    if TRACE:
        _install_profshim()
    nc = _build_bass(f8_seg_rows, NBF)
    res = run_bass_kernel_spmd(nc, in_maps, list(range(NCORES)), trace=TRACE)
    last_exec_time_ns = res.exec_time_ns
    if res.instructions_and_trace:
        last_trace_path = res.instructions_and_trace[1]

    Ys = np.stack([res.results[c]["y"] for c in range(NCORES)]).astype(np.float32)

    # Combine: out[b] = sum_k w_k * (Y/scale + b_e); duplicated fp8 pairs
    # average their two rows via two w/2 contributions.
    barange = np.arange(B)
    descale = np.float32(1.0 / (SX * SW))
    out = np.zeros((B, H), dtype=np.float32)
    for k in range(TOPK):
        ek = topk_idx[:, k]
        wk = topk_w[:, k]
        dmask = has_dup[ek, barange]
        w_eff = np.where(dmask, wk * 0.5, wk)
        yv = Ys[core_of[ek, barange], pos_of[ek, barange], :]
        sc = np.where(is8_of[ek, barange], w_eff * descale, w_eff).astype(np.float32)
        out += sc[:, None] * yv + wk[:, None] * expert_b[ek]
        di = np.nonzero(dmask)[0]
        if len(di):
            ekd = ek[di]
            yd = Ys[dup_core[ekd, di], dup_pos[ekd, di], :]
            out[di] += (wk[di] * 0.5 * descale)[:, None] * yd
    return out


# revision 29
# speedup vs baseline: 1.1649x; 1.1649x over previous
"""MoE layer (B=8192, D=2048, H=2048, E=8, top-2) on 8 TRN2 NeuronCores.

Strategy: expert-parallel with host-side routing + PER-PAIR MIXED PRECISION.
kernel() receives the FULL inputs on host, so the dispatch/combine all-to-all
is simply the sharding step:

  1. Gating (0.2% of FLOPs) on host with jax-CPU, bit-matching the
     reference's `x @ gate_W.T + gate_b` -> top_k -> softmax.
  2. Per expert, its T_BF*128 highest-gate-weight (token, expert) pairs run
     in bf16; the rest run in fp8 e4m3 with MatmulPerfMode.DoubleRow (2x PE
     throughput, 157 TF/s). Host-sim rel-err of this split is 1.64e-2 vs
     the 2e-2 gate (plain-fp8 everything would be 3.3e-2 — fails; all-bf16
     is 2x slower at 1.8x the HW time). fp8 scales align the uniform W
     distribution with the e4m3 grid (~15% less quant error than power-of-2
     scaling); the descale folds into the host combine.
  3. bf16 side: core e computes expert e's 1024 pairs in one 1024-row
     segment — zero padding. fp8 side: per-expert remainders are split into
     128-row blocks and best-fit packed into two segments per core
     (SPMD: all cores share one program, so segment shapes must match
     across cores). Leftover fp8 slot rows are filled with antithetically-
     quantized DUPLICATES of the highest-weight fp8 pairs; the combine
     averages each pair's two copies, cancelling ~75% of x-quantization
     error variance for those pairs at zero device cost.
  4. Segment order [fp8, fp8, bf16]: fp8 first tiles are half the bytes, so
     the PE starts earlier. More than 3 segments loses: each segment
     streams its own W copy and the extra DMA stalls the PE (measured).
     fp32 PSUM accumulate; bf16 output evicted via the vector engine
     (halves y write-back vs fp32).
  5. Host combine: out[b] = sum_k w_k[b] * (Y_pair[row(pair)]/scale(pair)
     + b_{e_k(b)}).
"""

import numpy as np

B, D, H, E, TOPK = 8192, 2048, 2048, 8, 2
NCORES = 8

T_BF = 8          # bf16 blocks (x128 rows) per expert = per core
SX = 24.0         # x fp8 scale (randn -> +-131, e4m3 normal range)
SW = 2976.0       # W fp8 scale (U(+-0.0221) -> +-65.8, grid-aligned)
# Dummies trip the HAM clock monitor AND bridge the PE queue until the
# first real matmul's tiles arrive (~17us for the custom fp8 head below):
# any >3.4us post-trip idle re-throttles the PE to 1.2GHz (measured as
# ~8.8us of 376-661ns matmuls when 48 dummies ended at 11.3us). 96
# dummies end ~17.2us, matching tile readiness.
WARMUP_MM = 96

# test.py flips TRACE to profile HW exec time; grading leaves it False.
TRACE = False
last_exec_time_ns = None
last_trace_path = None


def _routing(x, gate_W, gate_b):
    """Reference-exact gating on jax-CPU: logits -> top_k -> softmax."""
    import jax
    import jax.numpy as jnp

    with jax.default_device(jax.devices("cpu")[0]):
        logits = jnp.asarray(x) @ jnp.asarray(gate_W).T + jnp.asarray(gate_b)
        topk_vals, topk_idx = jax.lax.top_k(logits, TOPK)
        topk_w = jax.nn.softmax(topk_vals, axis=1)
    return np.asarray(topk_idx), np.asarray(topk_w, dtype=np.float32)


def _ok(b):
    # Segment block counts divisible by 3 or 4 keep matmul_tile_kernel's
    # M_TILE at 384/512 (vs 128), avoiding extra W re-streaming DMA.
    return b > 0 and (b % 3 == 0 or b % 4 == 0)


def _pack(block_counts):
    """Pack per-group block counts into identical per-core segments.

    Returns (seg_blocks, pieces) with pieces[g] = [(core, seg, blocks,
    bin_blocks)]; each (core, seg) bin holds at most one group, pieces fill
    bins from the start (partial fills leave slack at the bin end, which
    the dispatch fills with duplicated rows).
    """
    total = sum(block_counts)
    if total == 0:
        return [], [[] for _ in block_counts]
    t0 = -(-total // 8)
    for T in range(t0, t0 + 5):
        schemes = []
        if _ok(T):
            schemes.append([T])
        schemes += [
            [a, T - a] for a in range(T - 1, T // 2 - 1, -1) if _ok(a) and _ok(T - a)
        ]
        for seg_blocks in schemes:
            bins = []
            for c in range(8):
                for j, bcap in enumerate(seg_blocks):
                    bins.append([bcap, c, j])
            pieces = [[] for _ in block_counts]
            feasible = True
            for g in sorted(range(len(block_counts)), key=lambda g: -block_counts[g]):
                rem = block_counts[g]
                while rem > 0 and bins:
                    bins.sort(key=lambda s: -s[0])
                    if rem >= bins[0][0]:
                        pick, take = bins[0], bins[0][0]
                    else:
                        pick = min(
                            (s for s in bins if s[0] >= rem), key=lambda s: s[0]
                        )
                        take = rem
                    pieces[g].append((pick[1], pick[2], take, pick[0]))
                    rem -= take
                    bins.remove(pick)
                if rem > 0:
                    feasible = False
                    break
            if feasible:
                return seg_blocks, pieces
    raise RuntimeError("fp8 packing failed")


def _build_bass(f8_seg_rows, bf_rows):
    """One Bass program, SPMD across cores. Hand-rolled fp8 DoubleRow
    segments first, then the bf16 segment via matmul_tile_kernel. y rows
    follow segment order; bf16 dtype.

    The custom fp8 section fixes what matmul_tile_kernel can't: W8 streams
    exactly once (no per-M-tile re-streaming), and x8 arrives from a
    host-pre-tiled layout (128 descriptors of 2KB per tile instead of 2048
    of 128B), so the fp8 phase stops saturating the DMA queues and stalling
    the PE (measured: 6.7-12us of PE gaps -> ~2us).
    """
    import concourse.bacc as bacc
    import concourse.mybir as mybir
    import concourse.tile as tile
    from concourse.kernels.tile_matmul import matmul_tile_kernel

    P, KO, NB = 128, D // 128, H // 512
    C8 = sum(f8_seg_rows)
    M8 = C8 // P
    C = C8 + bf_rows
    f8, bf16, f32 = mybir.dt.float8e4, mybir.dt.bfloat16, mybir.dt.float32
    nc = bacc.Bacc("TRN2", target_bir_lowering=False)
    # x8t[p, m, ko, i] = x8[ko*128 + p, m*128 + i] — each SBUF tile
    # [128, KO, 128] is 2KB-contiguous per partition in DRAM
    x8t = nc.dram_tensor("x8t", [P, M8, KO, P], f8, kind="ExternalInput")
    xTb = nc.dram_tensor("xTb", [D, bf_rows], bf16, kind="ExternalInput")
    w8s = [
        nc.dram_tensor(f"w8_{j}", [D, H], f8, kind="ExternalInput")
        for j in range(len(f8_seg_rows))
    ]
    wb = nc.dram_tensor("wb", [D, H], bf16, kind="ExternalInput")
    y = nc.dram_tensor("y", [C, H], bf16, kind="ExternalOutput")
    with tile.TileContext(nc) as tc:
        # PE warm-up: tiny matmuls with no DMA deps run during the initial
        # tile-fill window, tripping the HAM activity monitor so the real
        # matmuls run at 2.4 GHz, and bridging the PE queue so it can't
        # re-throttle before the first real matmul.
        with (
            tc.tile_pool(name="warm", bufs=1) as warm,
            tc.tile_pool(name="warmp", bufs=1, space="PSUM") as warmp,
        ):
            wa = warm.tile([128, 128], bf16)
            nc.vector.memset(wa[:], 0.0)
            pts = [
                warmp.tile([128, 128], f32, name=f"wp{i}", tag=f"wp{i}")
                for i in range(4)
            ]
            for i in range(WARMUP_MM):
                nc.tensor.matmul(pts[i % 4][:], wa[:], wa[:], start=True, stop=True)

        # ---- custom fp8 DoubleRow section ----
        with (
            tc.tile_pool(name="w8pool", bufs=2) as wpool,
            tc.tile_pool(name="x8pool", bufs=1) as xpool,
            tc.tile_pool(name="y8pool", bufs=3) as ypool,
            tc.tile_pool(name="ps8", bufs=2, space="PSUM") as pspool,
        ):
            # x tiles become SBUF-resident (2.6MB total), loaded lazily in
            # each segment's first n-sweep so only x[0]+w[0] gate the first
            # matmul. x on the sync queue, steady-state weights on gpsimd,
            # so the two streams don't serialize on one issue queue.
            xt = [None] * M8
            mg = 0  # global m-chunk index == y row block in the fp8 region
            for j, S in enumerate(f8_seg_rows):
                wr = w8s[j].rearrange("(ko p) h -> p ko h", p=P)
                M = S // P
                for n in range(NB):
                    if n == 0:
                        xt[mg] = xpool.tile(
                            [P, KO, P], f8, name=f"x8_{mg}", tag=f"x8_{mg}"
                        )
                        if j == 0:
                            # split the very first x tile across two DMA
                            # queues: one dma_start = one queue at ~22GB/s,
                            # so a single 256KB load would gate the first
                            # matmul for ~12us
                            nc.sync.dma_start(
                                xt[mg][:, 0 : KO // 2, :], x8t[:, mg, 0 : KO // 2, :]
                            )
                            nc.sync.dma_start(
                                xt[mg][:, KO // 2 :, :], x8t[:, mg, KO // 2 :, :]
                            )
                        else:
                            nc.sync.dma_start(xt[mg][:], x8t[:, mg, :, :])
                    wt = []
                    for g in range(4):  # 4 ko-groups of 4 -> 256KB chunks
                        t = wpool.tile(
                            [P, 4, 512], f8, name=f"w8_{j}_{n}_{g}", tag=f"w8g_{g}"
                        )
                        sl = wr[:, g * 4 : (g + 1) * 4, n * 512 : (n + 1) * 512]
                        if j == 0 and n == 0:
                            # first W group on sync in 128KB halves: 8 issues
                            # land on 8 queues in parallel (~6us) instead of
                            # 4 serial 256KB gpsimd loads (~12us + dispatch)
                            nc.sync.dma_start(t[:, 0:2, :], sl[:, 0:2, :])
                            nc.sync.dma_start(t[:, 2:4, :], sl[:, 2:4, :])
                        else:
                            nc.gpsimd.dma_start(t[:], sl)
                        wt.append(t)
                    for m in range(M):
                        if n == 0 and m > 0:
                            xt[mg + m] = xpool.tile(
                                [P, KO, P], f8, name=f"x8_{mg + m}", tag=f"x8_{mg + m}"
                            )
                            nc.sync.dma_start(xt[mg + m][:], x8t[:, mg + m, :, :])
                        ps = pspool.tile([P, 512], f32, tag="ps8")
                        for k2 in range(KO // 2):  # 8 DoubleRow matmuls, K=256
                            ko = 2 * k2
                            nc.tensor.matmul(
                                ps[:],
                                xt[mg + m][:, ko : ko + 2, :],
                                wt[ko // 4][:, ko % 4 : ko % 4 + 2, :],
                                start=(k2 == 0),
                                stop=(k2 == KO // 2 - 1),
                                perf_mode=mybir.MatmulPerfMode.DoubleRow,
                            )
                        yt = ypool.tile([P, 512], bf16, tag="y8")
                        nc.vector.tensor_copy(yt[:], ps[:])
                        nc.sync.dma_start(
                            y[
                                (mg + m) * P : (mg + m + 1) * P,
                                n * 512 : (n + 1) * 512,
                            ],
                            yt[:],
                        )
                mg += M

        # ---- bf16 segment via the stock tile kernel ----
        matmul_tile_kernel(
            tc,
            xTb[:, :],
            wb[:],
            y[C8:, :],
            psum_evict_fn=lambda nc, psum, sbuf: nc.vector.tensor_copy(
                out=sbuf, in_=psum
            ),
        )
    nc.compile()
    return nc


def _install_profshim():
    """Register the NTFF profile hook trn_boot couldn't (image's antenv lacks
    axon_hooks) and stub the S3 artifact upload. Only needed when TRACE."""
    import sys
    import types

    import antenv

    if "antenv.axon_hooks" not in sys.modules:
        mod = types.ModuleType("antenv.axon_hooks")
        _hook = [None]
        mod.set_axon_ntff_profile_hook = lambda h: _hook.__setitem__(0, h)
        mod.get_axon_ntff_profile_hook = lambda: _hook[0]
        sys.modules["antenv.axon_hooks"] = mod
        antenv.axon_hooks = mod
        from trn_agent_boot.trn_boot import _ntff_profile_via_ctypes

        mod.set_axon_ntff_profile_hook(
            _ntff_profile_via_ctypes("/opt/axon/libaxon_pjrt.so")
        )
    import concourse.bass_utils as _bu

    _bu.upload_artifacts = lambda tmpdir: f"local:{tmpdir}"


def kernel(x, expert_W, expert_b, gate_W, gate_b):
    global last_exec_time_ns, last_trace_path
    import ml_dtypes

    from concourse.bass_utils import run_bass_kernel_spmd

    x = np.asarray(x, dtype=np.float32)
    expert_W = np.asarray(expert_W, dtype=np.float32)
    expert_b = np.asarray(expert_b, dtype=np.float32)
    gate_W = np.asarray(gate_W, dtype=np.float32)
    gate_b = np.asarray(gate_b, dtype=np.float32)

    topk_idx, topk_w = _routing(x, gate_W, gate_b)

    # per-expert token lists by gate weight desc; top T_BF*128 -> bf16
    lists = []
    for e in range(E):
        bb, kk = np.nonzero(topk_idx == e)
        ww = topk_w[bb, kk]
        o = np.argsort(-ww, kind="stable")
        lists.append(bb[o])
        assert len(bb) >= T_BF * 128, f"expert {e} has only {len(bb)} pairs"

    NBF = T_BF * 128
    counts_8 = [len(lists[e]) - NBF for e in range(E)]
    f8_blocks = [-(-n // 128) for n in counts_8]
    f8_seg, f8_pieces = _pack(f8_blocks)
    f8_seg_rows = [b * 128 for b in f8_seg]
    f8_off = np.concatenate([[0], np.cumsum(f8_seg_rows)]).astype(int)
    C8 = int(f8_off[-1])

    bf16 = ml_dtypes.bfloat16
    f8 = ml_dtypes.float8_e4m3
    xb = x.astype(bf16)  # one RTN cast, reused for all bf16 gathers
    x8 = (x * np.float32(SX)).astype(f8)
    # antithetic second quantization for duplicated rows: reflect x about its
    # first quantization so averaging the two copies cancels most of the
    # rounding error
    x8d = ((2.0 * x - x8.astype(np.float32) / np.float32(SX)) * np.float32(SX)).astype(
        f8
    )
    wbq = [np.ascontiguousarray(expert_W[e].T.astype(bf16)) for e in range(E)]
    w8q = [
        np.ascontiguousarray((expert_W[e].T * np.float32(SW)).astype(f8))
        for e in range(E)
    ]

    xT8s = [np.zeros((D, C8), dtype=f8) for _ in range(NCORES)]
    xTbs = [np.empty((D, NBF), dtype=bf16) for _ in range(NCORES)]
    seg8_w = [[0] * len(f8_seg) for _ in range(NCORES)]
    core_of = np.zeros((E, B), dtype=np.int64)
    pos_of = np.zeros((E, B), dtype=np.int64)
    is8_of = np.zeros((E, B), dtype=bool)
    dup_core = np.zeros((E, B), dtype=np.int64)
    dup_pos = np.zeros((E, B), dtype=np.int64)
    has_dup = np.zeros((E, B), dtype=bool)

    for e in range(E):
        toks = lists[e]
        tb, t8 = toks[:NBF], toks[NBF:]
        xTbs[e][:, :] = xb[tb].T  # bf16: expert e lives on core e, full slot
        core_of[e, tb] = e
        pos_of[e, tb] = C8 + np.arange(NBF)
        cum = 0
        dup_used = 0
        for c, j, blk, bin_blk in f8_pieces[e]:
            cap = bin_blk * 128  # fill the whole bin; slack becomes dups
            tkn = t8[cum : cum + blk * 128]
            lo = int(f8_off[j])
            xT8s[c][:, lo : lo + len(tkn)] = x8[tkn].T
            seg8_w[c][j] = e
            core_of[e, tkn] = c
            pos_of[e, tkn] = lo + np.arange(len(tkn))
            is8_of[e, tkn] = True
            slack = cap - len(tkn)
            if slack > 0:
                # fill leftover rows with antithetic duplicates of the
                # highest-weight fp8 pairs; combine averages the two copies
                dsel = t8[dup_used : dup_used + slack]
                if len(dsel):
                    xT8s[c][:, lo + len(tkn) : lo + len(tkn) + len(dsel)] = x8d[dsel].T
                    dup_core[e, dsel] = c
                    dup_pos[e, dsel] = lo + len(tkn) + np.arange(len(dsel))
                    has_dup[e, dsel] = True
                    dup_used += len(dsel)
            cum += blk * 128

    M8 = C8 // 128
    in_maps = []
    for c in range(NCORES):
        # pre-tile x8 for the custom section: [p, m, ko, i] so each SBUF
        # tile is 2KB-contiguous per partition
        x8tc = np.ascontiguousarray(
            xT8s[c].reshape(D // 128, 128, M8, 128).transpose(1, 2, 0, 3)
        )
        mm = {"x8t": x8tc, "xTb": xTbs[c], "wb": wbq[c]}
        for j in range(len(f8_seg)):
            mm[f"w8_{j}"] = w8q[seg8_w[c][j]]
        in_maps.append(mm)

    if TRACE:
        _install_profshim()
    nc = _build_bass(f8_seg_rows, NBF)
    res = run_bass_kernel_spmd(nc, in_maps, list(range(NCORES)), trace=TRACE)
    last_exec_time_ns = res.exec_time_ns
    if res.instructions_and_trace:
        last_trace_path = res.instructions_and_trace[1]

    Ys = np.stack([res.results[c]["y"] for c in range(NCORES)]).astype(np.float32)

    # Combine: out[b] = sum_k w_k * (Y/scale + b_e); duplicated fp8 pairs
    # average their two rows via two w/2 contributions.
    barange = np.arange(B)
    descale = np.float32(1.0 / (SX * SW))
    out = np.zeros((B, H), dtype=np.float32)
    for k in range(TOPK):
        ek = topk_idx[:, k]
        wk = topk_w[:, k]
        dmask = has_dup[ek, barange]
        w_eff = np.where(dmask, wk * 0.5, wk)
        yv = Ys[core_of[ek, barange], pos_of[ek, barange], :]
        sc = np.where(is8_of[ek, barange], w_eff * descale, w_eff).astype(np.float32)
        out += sc[:, None] * yv + wk[:, None] * expert_b[ek]
        di = np.nonzero(dmask)[0]
        if len(di):
            ekd = ek[di]
            yd = Ys[dup_core[ekd, di], dup_pos[ekd, di], :]
            out[di] += (wk[di] * 0.5 * descale)[:, None] * yd
    return out


# revision 30
# speedup vs baseline: 1.1855x; 1.0177x over previous
"""MoE layer (B=8192, D=2048, H=2048, E=8, top-2) on 8 TRN2 NeuronCores.

Strategy: expert-parallel with host-side routing + PER-PAIR MIXED PRECISION.
kernel() receives the FULL inputs on host, so the dispatch/combine all-to-all
is simply the sharding step:

  1. Gating (0.2% of FLOPs) on host with jax-CPU, bit-matching the
     reference's `x @ gate_W.T + gate_b` -> top_k -> softmax.
  2. Per expert, its T_BF*128 highest-gate-weight (token, expert) pairs run
     in bf16; the rest run in fp8 e4m3 with MatmulPerfMode.DoubleRow (2x PE
     throughput, 157 TF/s). Host-sim rel-err of this split is 1.64e-2 vs
     the 2e-2 gate (plain-fp8 everything would be 3.3e-2 — fails; all-bf16
     is 2x slower at 1.8x the HW time). fp8 scales align the uniform W
     distribution with the e4m3 grid (~15% less quant error than power-of-2
     scaling); the descale folds into the host combine.
  3. bf16 side: core e computes expert e's 1024 pairs in one 1024-row
     segment — zero padding. fp8 side: per-expert remainders are split into
     128-row blocks and best-fit packed into two segments per core
     (SPMD: all cores share one program, so segment shapes must match
     across cores). Leftover fp8 slot rows are filled with antithetically-
     quantized DUPLICATES of the highest-weight fp8 pairs; the combine
     averages each pair's two copies, cancelling ~75% of x-quantization
     error variance for those pairs at zero device cost.
  4. Segment order [fp8, fp8, bf16]: fp8 first tiles are half the bytes, so
     the PE starts earlier (the first call also uses a 256-row K tile to
     shrink the critical first transfers). More than 3 segments loses:
     each segment streams its own W copy and the extra DMA stalls the PE
     (measured). fp32 PSUM accumulate; bf16 output evicted via the vector
     engine (halves y write-back vs fp32).
  5. Host combine: out[b] = sum_k w_k[b] * (Y_pair[row(pair)]/scale(pair)
     + b_{e_k(b)}).
"""

import numpy as np

B, D, H, E, TOPK = 8192, 2048, 2048, 8, 2
NCORES = 8

T_BF = 8          # bf16 blocks (x128 rows) per expert = per core
SX = 24.0         # x fp8 scale (randn -> +-131, e4m3 normal range)
SW = 2976.0       # W fp8 scale (U(+-0.0221) -> +-65.8, grid-aligned)
# 48 dummies (~5.2us) trip the HAM clock monitor AND bridge the PE queue
# until the first real matmul's tiles arrive (~12.5us): fewer dummies leave
# an idle gap that re-throttles the PE to 1.2GHz (measured: 28 dummies ->
# first ~100 matmuls at 376ns instead of 216ns).
WARMUP_MM = 48

# test.py flips TRACE to profile HW exec time; grading leaves it False.
TRACE = False
last_exec_time_ns = None
last_trace_path = None


def _routing(x, gate_W, gate_b):
    """Reference-exact gating on jax-CPU: logits -> top_k -> softmax."""
    import jax
    import jax.numpy as jnp

    with jax.default_device(jax.devices("cpu")[0]):
        logits = jnp.asarray(x) @ jnp.asarray(gate_W).T + jnp.asarray(gate_b)
        topk_vals, topk_idx = jax.lax.top_k(logits, TOPK)
        topk_w = jax.nn.softmax(topk_vals, axis=1)
    return np.asarray(topk_idx), np.asarray(topk_w, dtype=np.float32)


def _ok(b):
    # Segment block counts divisible by 3 or 4 keep matmul_tile_kernel's
    # M_TILE at 384/512 (vs 128), avoiding extra W re-streaming DMA.
    return b > 0 and (b % 3 == 0 or b % 4 == 0)


def _pack(block_counts):
    """Pack per-group block counts into identical per-core segments.

    Returns (seg_blocks, pieces) with pieces[g] = [(core, seg, blocks,
    bin_blocks)]; each (core, seg) bin holds at most one group, pieces fill
    bins from the start (partial fills leave slack at the bin end, which
    the dispatch fills with duplicated rows).
    """
    total = sum(block_counts)
    if total == 0:
        return [], [[] for _ in block_counts]
    t0 = -(-total // 8)
    for T in range(t0, t0 + 5):
        schemes = []
        if _ok(T):
            schemes.append([T])
        schemes += [
            [a, T - a] for a in range(T - 1, T // 2 - 1, -1) if _ok(a) and _ok(T - a)
        ]
        for seg_blocks in schemes:
            bins = []
            for c in range(8):
                for j, bcap in enumerate(seg_blocks):
                    bins.append([bcap, c, j])
            pieces = [[] for _ in block_counts]
            feasible = True
            for g in sorted(range(len(block_counts)), key=lambda g: -block_counts[g]):
                rem = block_counts[g]
                while rem > 0 and bins:
                    bins.sort(key=lambda s: -s[0])
                    if rem >= bins[0][0]:
                        pick, take = bins[0], bins[0][0]
                    else:
                        pick = min(
                            (s for s in bins if s[0] >= rem), key=lambda s: s[0]
                        )
                        take = rem
                    pieces[g].append((pick[1], pick[2], take, pick[0]))
                    rem -= take
                    bins.remove(pick)
                if rem > 0:
                    feasible = False
                    break
            if feasible:
                return seg_blocks, pieces
    raise RuntimeError("fp8 packing failed")


def _build_bass(f8_seg_rows, bf_rows):
    """One Bass program, SPMD across cores. fp8 (DoubleRow) segments first,
    then the bf16 segment. y rows follow segment order; bf16 dtype."""
    import concourse.bacc as bacc
    import concourse.mybir as mybir
    import concourse.tile as tile
    from concourse.kernels.tile_matmul import matmul_tile_kernel

    C8 = sum(f8_seg_rows)
    C = C8 + bf_rows
    nc = bacc.Bacc("TRN2", target_bir_lowering=False)
    xT8 = nc.dram_tensor("xT8", [D, C8], mybir.dt.float8e4, kind="ExternalInput")
    xTb = nc.dram_tensor("xTb", [D, bf_rows], mybir.dt.bfloat16, kind="ExternalInput")
    w8s = [
        nc.dram_tensor(f"w8_{j}", [D, H], mybir.dt.float8e4, kind="ExternalInput")
        for j in range(len(f8_seg_rows))
    ]
    wb = nc.dram_tensor("wb", [D, H], mybir.dt.bfloat16, kind="ExternalInput")
    y = nc.dram_tensor("y", [C, H], mybir.dt.bfloat16, kind="ExternalOutput")
    with tile.TileContext(nc) as tc:
        # PE warm-up: tiny matmuls with no DMA deps run during the initial
        # tile-fill window, tripping the HAM activity monitor (4096-cycle
        # window) so the real matmuls start at 2.4 GHz instead of the cold
        # 1.2 GHz, and bridging the idle gap so it can't re-throttle before
        # the first real matmul.
        with (
            tc.tile_pool(name="warm", bufs=1) as warm,
            tc.tile_pool(name="warmp", bufs=1, space="PSUM") as warmp,
        ):
            wa = warm.tile([128, 128], mybir.dt.bfloat16)
            nc.vector.memset(wa[:], 0.0)
            pts = [
                warmp.tile([128, 128], mybir.dt.float32, name=f"wp{i}", tag=f"wp{i}")
                for i in range(4)
            ]
            for i in range(WARMUP_MM):
                nc.tensor.matmul(pts[i % 4][:], wa[:], wa[:], start=True, stop=True)

        evict = lambda nc, psum, sbuf: nc.vector.tensor_copy(out=sbuf, in_=psum)
        off = 0
        for j, s in enumerate(f8_seg_rows):
            matmul_tile_kernel(
                tc,
                xT8[:, off : off + s],
                w8s[j][:],
                y[off : off + s, :],
                psum_evict_fn=evict,
            )
            off += s
        matmul_tile_kernel(
            tc,
            xTb[:, :],
            wb[:],
            y[C8:, :],
            psum_evict_fn=evict,
        )
    nc.compile()
    return nc


def _install_profshim():
    """Register the NTFF profile hook trn_boot couldn't (image's antenv lacks
    axon_hooks) and stub the S3 artifact upload. Only needed when TRACE."""
    import sys
    import types

    import antenv

    if "antenv.axon_hooks" not in sys.modules:
        mod = types.ModuleType("antenv.axon_hooks")
        _hook = [None]
        mod.set_axon_ntff_profile_hook = lambda h: _hook.__setitem__(0, h)
        mod.get_axon_ntff_profile_hook = lambda: _hook[0]
        sys.modules["antenv.axon_hooks"] = mod
        antenv.axon_hooks = mod
        from trn_agent_boot.trn_boot import _ntff_profile_via_ctypes

        mod.set_axon_ntff_profile_hook(
            _ntff_profile_via_ctypes("/opt/axon/libaxon_pjrt.so")
        )
    import concourse.bass_utils as _bu

    _bu.upload_artifacts = lambda tmpdir: f"local:{tmpdir}"


def kernel(x, expert_W, expert_b, gate_W, gate_b):
    global last_exec_time_ns, last_trace_path
    import ml_dtypes

    from concourse.bass_utils import run_bass_kernel_spmd

    x = np.asarray(x, dtype=np.float32)
    expert_W = np.asarray(expert_W, dtype=np.float32)
    expert_b = np.asarray(expert_b, dtype=np.float32)
    gate_W = np.asarray(gate_W, dtype=np.float32)
    gate_b = np.asarray(gate_b, dtype=np.float32)

    topk_idx, topk_w = _routing(x, gate_W, gate_b)

    # per-expert token lists by gate weight desc; top T_BF*128 -> bf16
    lists = []
    for e in range(E):
        bb, kk = np.nonzero(topk_idx == e)
        ww = topk_w[bb, kk]
        o = np.argsort(-ww, kind="stable")
        lists.append(bb[o])
        assert len(bb) >= T_BF * 128, f"expert {e} has only {len(bb)} pairs"

    NBF = T_BF * 128
    counts_8 = [len(lists[e]) - NBF for e in range(E)]
    f8_blocks = [-(-n // 128) for n in counts_8]
    f8_seg, f8_pieces = _pack(f8_blocks)
    f8_seg_rows = [b * 128 for b in f8_seg]
    f8_off = np.concatenate([[0], np.cumsum(f8_seg_rows)]).astype(int)
    C8 = int(f8_off[-1])

    bf16 = ml_dtypes.bfloat16
    f8 = ml_dtypes.float8_e4m3
    xb = x.astype(bf16)  # one RTN cast, reused for all bf16 gathers
    x8 = (x * np.float32(SX)).astype(f8)
    # antithetic second quantization for duplicated rows: reflect x about its
    # first quantization so averaging the two copies cancels most of the
    # rounding error
    x8d = ((2.0 * x - x8.astype(np.float32) / np.float32(SX)) * np.float32(SX)).astype(
        f8
    )
    wbq = [np.ascontiguousarray(expert_W[e].T.astype(bf16)) for e in range(E)]
    w8q = [
        np.ascontiguousarray((expert_W[e].T * np.float32(SW)).astype(f8))
        for e in range(E)
    ]

    xT8s = [np.zeros((D, C8), dtype=f8) for _ in range(NCORES)]
    xTbs = [np.empty((D, NBF), dtype=bf16) for _ in range(NCORES)]
    seg8_w = [[0] * len(f8_seg) for _ in range(NCORES)]
    core_of = np.zeros((E, B), dtype=np.int64)
    pos_of = np.zeros((E, B), dtype=np.int64)
    is8_of = np.zeros((E, B), dtype=bool)
    dup_core = np.zeros((E, B), dtype=np.int64)
    dup_pos = np.zeros((E, B), dtype=np.int64)
    has_dup = np.zeros((E, B), dtype=bool)

    for e in range(E):
        toks = lists[e]
        tb, t8 = toks[:NBF], toks[NBF:]
        xTbs[e][:, :] = xb[tb].T  # bf16: expert e lives on core e, full slot
        core_of[e, tb] = e
        pos_of[e, tb] = C8 + np.arange(NBF)
        cum = 0
        dup_used = 0
        for c, j, blk, bin_blk in f8_pieces[e]:
            cap = bin_blk * 128  # fill the whole bin; slack becomes dups
            tkn = t8[cum : cum + blk * 128]
            lo = int(f8_off[j])
            xT8s[c][:, lo : lo + len(tkn)] = x8[tkn].T
            seg8_w[c][j] = e
            core_of[e, tkn] = c
            pos_of[e, tkn] = lo + np.arange(len(tkn))
            is8_of[e, tkn] = True
            slack = cap - len(tkn)
            if slack > 0:
                # fill leftover rows with antithetic duplicates of the
                # highest-weight fp8 pairs; combine averages the two copies
                dsel = t8[dup_used : dup_used + slack]
                if len(dsel):
                    xT8s[c][:, lo + len(tkn) : lo + len(tkn) + len(dsel)] = x8d[dsel].T
                    dup_core[e, dsel] = c
                    dup_pos[e, dsel] = lo + len(tkn) + np.arange(len(dsel))
                    has_dup[e, dsel] = True
                    dup_used += len(dsel)
            cum += blk * 128

    in_maps = []
    for c in range(NCORES):
        mm = {"xT8": xT8s[c], "xTb": xTbs[c], "wb": wbq[c]}
        for j in range(len(f8_seg)):
            mm[f"w8_{j}"] = w8q[seg8_w[c][j]]
        in_maps.append(mm)

    if TRACE:
        _install_profshim()
    nc = _build_bass(f8_seg_rows, NBF)
    res = run_bass_kernel_spmd(nc, in_maps, list(range(NCORES)), trace=TRACE)
    last_exec_time_ns = res.exec_time_ns
    if res.instructions_and_trace:
        last_trace_path = res.instructions_and_trace[1]

    Ys = np.stack([res.results[c]["y"] for c in range(NCORES)]).astype(np.float32)

    # Combine: out[b] = sum_k w_k * (Y/scale + b_e); duplicated fp8 pairs
    # average their two rows via two w/2 contributions.
    barange = np.arange(B)
    descale = np.float32(1.0 / (SX * SW))
    out = np.zeros((B, H), dtype=np.float32)
    for k in range(TOPK):
        ek = topk_idx[:, k]
        wk = topk_w[:, k]
        dmask = has_dup[ek, barange]
        w_eff = np.where(dmask, wk * 0.5, wk)
        yv = Ys[core_of[ek, barange], pos_of[ek, barange], :]
        sc = np.where(is8_of[ek, barange], w_eff * descale, w_eff).astype(np.float32)
        out += sc[:, None] * yv + wk[:, None] * expert_b[ek]
        di = np.nonzero(dmask)[0]
        if len(di):
            ekd = ek[di]
            yd = Ys[dup_core[ekd, di], dup_pos[ekd, di], :]
            out[di] += (wk[di] * 0.5 * descale)[:, None] * yd
    return out
